# revision 21
# baseline (speedup 1.0000x reference)
"""Trainium2 Bass kernel for DCTEncoderLayer — "stationary swap" v3.8.

Computes, for rgb_images_batch [32, 3, 512, 512] f32:
  ycbcr' = 2*rgb_to_ycbcr(rgb) - 1             (per-pixel 3x3 channel mix)
  32x32 block DCT per channel, coefficients scaled by (2/32)*c_u*c_v,
  output [32, 3*1024, 16, 16] with the frequency axis sorted by |(v,u)|.

Pure data parallel over batch: 4 images per NeuronCore, 8 cores.

The 2D DCT is separable: coeff = Cs @ block @ Cs.T with
Cs[v,y] = cos((2y+1)v*pi/64) * c_v / 4; the YCbCr mix (linear; feeding
the device rgb-0.5 makes the affine offset exact) is folded into the
stage-1 weights.  Stage 1 runs with the IMAGE as the PE stationary
operand:

    t1T[x, (c,v)] = img_chunk[(c',y), x].T @ W1[(c',y), (c,v)]

which lands the stage-1 result already transposed (x on partitions), so
no stream-transpose is needed anywhere.  Stage 2 is a single
128-partition block-diagonal DCT along x':

    out[(gxl,u), (c,v)] = W2bd[(gxl,x'), (gxl,u)].T @ t1s[(gxl,x'), (c,v)]

Performance structure (measured on TRN2):
  - DMAs carry ~350-600ns fixed cost, so input DMAs batch 4 block-rows
    (host lays x out so each partition reads 4KB contiguously) and
    output DMAs batch 8 block-rows; outputs issue from the otherwise
    idle GpSimd DGE queue so they never queue behind inputs on Sync.
  - The two PSUM->SBUF evacuations pace the kernel (only ACT and DVE
    can touch PSUM): cast1 (stage-1 results, fp16) lives on ACT only
    and cast2 (stage-2 results, fp16, two block-rows at a time) on DVE
    only — alternating engines would queue cast1(i) behind cast2(i-1)
    and serialize the PE->cast1->PE->cast2 chain.
  - Stage-1's four chunk matmuls pack contiguously at 96-column offsets
    inside one PSUM bank (hardware zeroes per written byte on
    start=True, verified on HW), keeping cast1 a flat 2D copy.
  - A short garbage-matmul warm-up during the initial DMA fill ramps
    the PE out of its low p-state.
  - fp16 everywhere off-chip halves both DMA directions; rel err vs the
    f64 reference is ~5.8e-4 (dominated by fp16 input rounding).

The device writes [8, 128, 3072] fp16 per core; the host upcasts,
permutes axes and applies the frequency sort.
"""

import os
import sys

try:
    import concourse.bass  # noqa: F401
except ImportError:
    sys.path.insert(0, "/opt/trn_rl_repo")

import numpy as np

import concourse.bacc as bacc
import concourse.bass as bass
import concourse.mybir as mybir
import concourse.tile as tile
from concourse.bass_utils import run_bass_kernel_spmd

F32 = mybir.dt.float32
F16 = mybir.dt.float16

BS = 32
N_CORES = 8
B_PER_CORE = 4
NH = 16
ITERS = B_PER_CORE * NH  # 64 block-rows per core
GROUPS = ITERS // 4      # 16 groups of 4 block-rows

_STATE = {}
LAST_RESULT = None


def _dct_mat():
    y = np.arange(BS)
    v = np.arange(BS)[:, None]
    c = np.cos((2 * y + 1) * v * np.pi / (2 * BS))
    c[0, :] *= 1.0 / np.sqrt(2.0)
    return c / 4.0


def _sort_idx():
    # must replicate the reference's argsort (default kind) exactly,
    # including its tie order for equal |(v,u)|
    mag = np.zeros((BS, BS), dtype=np.float64)
    for v in range(BS):
        for u in range(BS):
            mag[v, u] = np.linalg.norm(np.array([v, u], dtype=np.int64))
    return np.argsort(mag.reshape(-1))


def _constants():
    cs = _dct_mat()
    # rows (y', cb', cr') of the linear part of 2*rgb_to_ycbcr(rgb)-1, in (r,g,b)
    a2 = np.array(
        [
            [2 * 0.299, 2 * 0.587, 2 * 0.114],
            [2 * 0.564 * -0.299, 2 * 0.564 * -0.587, 2 * 0.564 * (1 - 0.114)],
            [2 * 0.713 * (1 - 0.299), 2 * 0.713 * -0.587, 2 * 0.713 * -0.114],
        ],
        np.float64,
    )
    w1 = np.zeros((96, 96))  # [(c', y), (c, v)]
    for cp in range(3):
        for c in range(3):
            w1[cp * 32 : (cp + 1) * 32, c * 32 : (c + 1) * 32] = a2[c, cp] * cs.T
    w2 = np.zeros((128, 128))  # [(gxl, x'), (gxl, u)] block diagonal over gxl
    for g in range(4):
        w2[g * 32 : (g + 1) * 32, g * 32 : (g + 1) * 32] = cs.T
    return w1.astype(np.float16), w2.astype(np.float16)


def _build_program():
    nc = bacc.Bacc(trn_type="TRN2")
    # host pre-groups 4 block-rows so each partition's 4KB is contiguous
    x = nc.dram_tensor("x", [GROUPS, 96, 4, 512], F16, kind="ExternalInput")
    w1 = nc.dram_tensor("w1", [96, 96], F16, kind="ExternalInput")
    w2 = nc.dram_tensor("w2", [128, 128], F16, kind="ExternalInput")
    out = nc.dram_tensor("out", [GROUPS // 2, 128, 3072], F16, kind="ExternalOutput")

    with tile.TileContext(nc) as tc:
        with (
            tc.tile_pool(name="const", bufs=1) as constp,
            tc.tile_pool(name="pin", bufs=4) as pin,
            tc.tile_pool(name="pmid", bufs=8) as pmid,
            tc.tile_pool(name="pout", bufs=3) as pout,
            tc.tile_pool(name="psA", bufs=2, space="PSUM") as psA,
            tc.tile_pool(name="psB", bufs=3, space="PSUM") as psB,
        ):
            w1s = constp.tile([96, 96], F16)
            w2s = constp.tile([128, 128], F16)
            # weights load on the ACT HWDGE queue so the PE's first real
            # matmul doesn't queue behind the Sync img prefetch burst
            nc.scalar.dma_start(w1s[:], w1[:])
            nc.scalar.dma_start(w2s[:], w2[:])
            # PE warm-up: ~3.5us of garbage matmuls during the DMA fill
            # phase ramps the PE out of its low p-state before real work
            spin_w = constp.tile([128, 128], F16)
            spin_x = constp.tile([128, 512], F16)
            nc.gpsimd.memset(spin_w[:], 0)
            nc.gpsimd.memset(spin_x[:], 0)
            warm = psA.tile([128, 384], F32, tag="t1T")
            for _ in range(6):
                nc.tensor.matmul(
                    warm[:], spin_w[:], spin_x[:, 0:384], start=True, stop=True
                )

            for g in range(GROUPS):
                img4 = pin.tile([96, 2048], F16, tag="img4")
                nc.sync.dma_start(
                    img4[:].rearrange("p (r x) -> p r x", r=4), x[g]
                )
                if g % 2 == 0:
                    osb8 = pout.tile([128, 3072], F16, tag="osb8")
                o2p = None
                for j in range(4):
                    # stage 1 (stationary swap): 4 chunk outputs packed
                    # contiguously at 96-col offsets — 1536B, all within one
                    # PSUM bank (HW zeroes per written byte on start=True)
                    t1T = psA.tile([128, 384], F32, tag="t1T")
                    for k in range(4):
                        nc.tensor.matmul(
                            t1T[:, k * 96 : (k + 1) * 96],
                            img4[:, j * 512 + k * 128 : j * 512 + (k + 1) * 128],
                            w1s[:],
                            start=True,
                            stop=True,
                        )
                    # cast1 (ACT only): flat contiguous copy -> fp16 [128,384].
                    # keeping cast1 off DVE avoids queueing it behind cast2,
                    # which would serialize the PE->cast1->PE->cast2 chain
                    t1s = pmid.tile([128, 384], F16, tag="t1s")
                    nc.scalar.copy(t1s[:], t1T[:])
                    # stage 2: one matmul; two iterations share a 2-bank tile
                    if j % 2 == 0:
                        o2p = psB.tile([128, 1024], F32, tag="o2p")
                    nc.tensor.matmul(
                        o2p[:, (j % 2) * 512 : (j % 2) * 512 + 384],
                        w2s[:],
                        t1s[:],
                        start=True,
                        stop=True,
                    )
                    # cast2 (DVE only) covers both halves once the pair is done
                    if j % 2 == 1:
                        base = (g % 2) * 1536 + (j - 1) * 384
                        csrc = o2p[:].rearrange("p (r s) -> p r s", r=2)[:, :, 0:384]
                        cdst = osb8[:, base : base + 768].rearrange(
                            "p (r s) -> p r s", r=2
                        )
                        nc.vector.tensor_copy(cdst, csrc)
                # one output DMA per 8 iterations, alternating GpSimd /
                # Sync queues (halves the SWDGE ring -> smaller final drain)
                if g % 2 == 1:
                    qeng = nc.gpsimd if (g // 2) % 2 == 0 else nc.sync
                    qeng.dma_start(out[g // 2], osb8[:])

    nc.finalize()
    return nc


def _get_program():
    if "nc" not in _STATE:
        _STATE["nc"] = _build_program()
        _STATE["consts"] = _constants()
        _STATE["sort_idx"] = _sort_idx()
    return _STATE["nc"]


def kernel(**inputs):
    global LAST_RESULT
    rgb = np.asarray(inputs["rgb_images_batch"], np.float32)
    assert rgb.shape == (N_CORES * B_PER_CORE, 3, 512, 512)
    B = N_CORES * B_PER_CORE
    xs = rgb.reshape(B, 3, NH, 32, 512).transpose(0, 2, 1, 3, 4)
    # centering makes the YCbCr affine offset vanish (row sums of the cb/cr
    # mix are 0 and the y row sums to 2 -> offset 2*0.5-1=0 per channel)
    xs = (np.ascontiguousarray(xs).reshape(B, NH, 96, 512)
          - np.float32(0.5)).astype(np.float16)
    # group 4 block-rows with the partition dim outermost: [B, g, 96, 4r, 512]
    xs = np.ascontiguousarray(xs.reshape(B, NH // 4, 4, 96, 512).transpose(0, 1, 3, 2, 4))
    xs = xs.reshape(B, NH // 4, 96, 4, 512)
    nc = _get_program()
    w1, w2 = _STATE["consts"]
    sort_idx = _STATE["sort_idx"]

    in_maps = [
        {
            "x": xs[c * B_PER_CORE : (c + 1) * B_PER_CORE].reshape(GROUPS, 96, 4, 512),
            "w1": w1,
            "w2": w2,
        }
        for c in range(N_CORES)
    ]
    trace = os.environ.get("KERNEL_TRACE", "0") == "1"
    res = run_bass_kernel_spmd(
        nc, in_maps, core_ids=list(range(N_CORES)), trace=trace
    )
    LAST_RESULT = res

    outs = []
    for c in range(N_CORES):
        dev = res.results[c]["out"].astype(np.float32)  # [8, 128, 3072]
        dev = dev.reshape(GROUPS // 2, 128, 8, 384).transpose(0, 2, 1, 3)
        dev = dev.reshape(ITERS, 128, 384)
        # [it=(b,br), p=(gxl,u), col=(kk, c, v)]
        a = dev.reshape(B_PER_CORE, NH, 4, 32, 4, 3, 32)  # b,br,gxl,u,kk,c,v
        a = a.transpose(0, 5, 6, 3, 1, 4, 2)  # b,c,v,u,br,kk,gxl
        a = np.ascontiguousarray(a).reshape(B_PER_CORE, 3, 1024, NH, NH)
        a = a[:, :, sort_idx, :, :]
        outs.append(a.reshape(B_PER_CORE, 3 * 1024, NH, NH))
    return np.concatenate(outs, axis=0)


# revision 23
# speedup vs baseline: 1.0606x; 1.0606x over previous
"""Trainium2 Bass kernel for DCTEncoderLayer — "stationary swap" v3.8.

Computes, for rgb_images_batch [32, 3, 512, 512] f32:
  ycbcr' = 2*rgb_to_ycbcr(rgb) - 1             (per-pixel 3x3 channel mix)
  32x32 block DCT per channel, coefficients scaled by (2/32)*c_u*c_v,
  output [32, 3*1024, 16, 16] with the frequency axis sorted by |(v,u)|.

Pure data parallel over batch: 4 images per NeuronCore, 8 cores.

The 2D DCT is separable: coeff = Cs @ block @ Cs.T with
Cs[v,y] = cos((2y+1)v*pi/64) * c_v / 4; the YCbCr mix (linear; feeding
the device rgb-0.5 makes the affine offset exact) is folded into the
stage-1 weights.  Stage 1 runs with the IMAGE as the PE stationary
operand:

    t1T[x, (c,v)] = img_chunk[(c',y), x].T @ W1[(c',y), (c,v)]

which lands the stage-1 result already transposed (x on partitions), so
no stream-transpose is needed anywhere.  Stage 2 is a single
128-partition block-diagonal DCT along x':

    out[(gxl,u), (c,v)] = W2bd[(gxl,x'), (gxl,u)].T @ t1s[(gxl,x'), (c,v)]

Performance structure (measured on TRN2):
  - DMAs carry ~350-600ns fixed cost, so input DMAs batch 4 block-rows
    (host lays x out so each partition reads 4KB contiguously) and
    output DMAs batch 8 block-rows; outputs issue from the otherwise
    idle GpSimd DGE queue so they never queue behind inputs on Sync.
  - The two PSUM->SBUF evacuations pace the kernel (only ACT and DVE
    can touch PSUM): cast1 (stage-1 results, fp16) lives on ACT only
    and cast2 (stage-2 results, fp16, two block-rows at a time) on DVE
    only — alternating engines would queue cast1(i) behind cast2(i-1)
    and serialize the PE->cast1->PE->cast2 chain.
  - Stage-1's four chunk matmuls pack contiguously at 96-column offsets
    inside one PSUM bank (hardware zeroes per written byte on
    start=True, verified on HW), keeping cast1 a flat 2D copy.
  - A short garbage-matmul warm-up during the initial DMA fill ramps
    the PE out of its low p-state.
  - fp16 everywhere off-chip halves both DMA directions; rel err vs the
    f64 reference is ~5.8e-4 (dominated by fp16 input rounding).

The device writes [8, 128, 3072] fp16 per core; the host upcasts,
permutes axes and applies the frequency sort.
"""

import os
import sys

try:
    import concourse.bass  # noqa: F401
except ImportError:
    sys.path.insert(0, "/opt/trn_rl_repo")

import numpy as np

import concourse.bacc as bacc
import concourse.bass as bass
import concourse.mybir as mybir
import concourse.tile as tile
from concourse.bass_utils import run_bass_kernel_spmd

F32 = mybir.dt.float32
F16 = mybir.dt.float16

BS = 32
N_CORES = 8
B_PER_CORE = 4
NH = 16
ITERS = B_PER_CORE * NH  # 64 block-rows per core
GROUPS = ITERS // 4      # 16 groups of 4 block-rows

_STATE = {}
LAST_RESULT = None


def _dct_mat():
    y = np.arange(BS)
    v = np.arange(BS)[:, None]
    c = np.cos((2 * y + 1) * v * np.pi / (2 * BS))
    c[0, :] *= 1.0 / np.sqrt(2.0)
    return c / 4.0


def _sort_idx():
    # must replicate the reference's argsort (default kind) exactly,
    # including its tie order for equal |(v,u)|
    mag = np.zeros((BS, BS), dtype=np.float64)
    for v in range(BS):
        for u in range(BS):
            mag[v, u] = np.linalg.norm(np.array([v, u], dtype=np.int64))
    return np.argsort(mag.reshape(-1))


def _constants():
    cs = _dct_mat()
    # rows (y', cb', cr') of the linear part of 2*rgb_to_ycbcr(rgb)-1, in (r,g,b)
    a2 = np.array(
        [
            [2 * 0.299, 2 * 0.587, 2 * 0.114],
            [2 * 0.564 * -0.299, 2 * 0.564 * -0.587, 2 * 0.564 * (1 - 0.114)],
            [2 * 0.713 * (1 - 0.299), 2 * 0.713 * -0.587, 2 * 0.713 * -0.114],
        ],
        np.float64,
    )
    w1 = np.zeros((96, 96))  # [(c', y), (c, v)]
    for cp in range(3):
        for c in range(3):
            w1[cp * 32 : (cp + 1) * 32, c * 32 : (c + 1) * 32] = a2[c, cp] * cs.T
    w2 = np.zeros((128, 128))  # [(gxl, x'), (gxl, u)] block diagonal over gxl
    for g in range(4):
        w2[g * 32 : (g + 1) * 32, g * 32 : (g + 1) * 32] = cs.T
    return w1.astype(np.float16), w2.astype(np.float16)


def _build_program():
    nc = bacc.Bacc(trn_type="TRN2")
    # host pre-groups 4 block-rows so each partition's 4KB is contiguous
    x = nc.dram_tensor("x", [GROUPS, 96, 4, 512], F16, kind="ExternalInput")
    w1 = nc.dram_tensor("w1", [96, 96], F16, kind="ExternalInput")
    w2 = nc.dram_tensor("w2", [128, 128], F16, kind="ExternalInput")
    out = nc.dram_tensor("out", [GROUPS // 2, 128, 3072], F16, kind="ExternalOutput")

    with tile.TileContext(nc) as tc:
        with (
            tc.tile_pool(name="const", bufs=1) as constp,
            tc.tile_pool(name="pin", bufs=4) as pin,
            tc.tile_pool(name="pmid", bufs=8) as pmid,
            tc.tile_pool(name="pout", bufs=3) as pout,
            tc.tile_pool(name="psA", bufs=4, space="PSUM") as psA,
            tc.tile_pool(name="psB", bufs=2, space="PSUM") as psB,
        ):
            w1s = constp.tile([96, 96], F16)
            w2s = constp.tile([128, 128], F16)
            # weights load on the ACT HWDGE queue so the PE's first real
            # matmul doesn't queue behind the Sync img prefetch burst
            nc.scalar.dma_start(w1s[:], w1[:])
            nc.scalar.dma_start(w2s[:], w2[:])
            # PE warm-up: ~3.5us of garbage matmuls during the DMA fill
            # phase ramps the PE out of its low p-state before real work
            spin_w = constp.tile([128, 128], F16)
            spin_x = constp.tile([128, 512], F16)
            nc.gpsimd.memset(spin_w[:], 0)
            nc.gpsimd.memset(spin_x[:], 0)
            warm = psA.tile([128, 384], F32, tag="t1T")
            for _ in range(8):
                nc.tensor.matmul(
                    warm[:], spin_w[:], spin_x[:, 0:384], start=True, stop=True
                )

            for g in range(GROUPS):
                img4 = pin.tile([96, 2048], F16, tag="img4")
                nc.sync.dma_start(
                    img4[:].rearrange("p (r x) -> p r x", r=4), x[g]
                )
                if g % 2 == 0:
                    osb8 = pout.tile([128, 3072], F16, tag="osb8")
                o2p = None
                for j in range(4):
                    # stage 1 (stationary swap): 4 chunk outputs packed
                    # contiguously at 96-col offsets — 1536B, all within one
                    # PSUM bank (HW zeroes per written byte on start=True)
                    t1T = psA.tile([128, 384], F32, tag="t1T")
                    for k in range(4):
                        nc.tensor.matmul(
                            t1T[:, k * 96 : (k + 1) * 96],
                            img4[:, j * 512 + k * 128 : j * 512 + (k + 1) * 128],
                            w1s[:],
                            start=True,
                            stop=True,
                        )
                    # cast1 (ACT only): flat contiguous copy -> fp16 [128,384].
                    # keeping cast1 off DVE avoids queueing it behind cast2,
                    # which would serialize the PE->cast1->PE->cast2 chain
                    t1s = pmid.tile([128, 384], F16, tag="t1s")
                    nc.scalar.copy(t1s[:], t1T[:])
                    # stage 2: one matmul; two iterations share a 2-bank tile
                    if j % 2 == 0:
                        o2p = psB.tile([128, 1024], F32, tag="o2p")
                    nc.tensor.matmul(
                        o2p[:, (j % 2) * 512 : (j % 2) * 512 + 384],
                        w2s[:],
                        t1s[:],
                        start=True,
                        stop=True,
                    )
                    # cast2 (DVE only) covers both halves once the pair is done
                    if j % 2 == 1:
                        base = (g % 2) * 1536 + (j - 1) * 384
                        csrc = o2p[:].rearrange("p (r s) -> p r s", r=2)[:, :, 0:384]
                        cdst = osb8[:, base : base + 768].rearrange(
                            "p (r s) -> p r s", r=2
                        )
                        nc.vector.tensor_copy(cdst, csrc)
                # one output DMA per 8 iterations, on the GpSimd DGE
                # queue; the last two go via Sync's HWDGE (idle by then) so
                # the SWDGE ring drains early, off the critical tail
                if g % 2 == 1:
                    qeng = nc.gpsimd if g < 12 else nc.sync
                    qeng.dma_start(out[g // 2], osb8[:])

    nc.finalize()
    return nc


def _get_program():
    if "nc" not in _STATE:
        _STATE["nc"] = _build_program()
        _STATE["consts"] = _constants()
        _STATE["sort_idx"] = _sort_idx()
    return _STATE["nc"]


def kernel(**inputs):
    global LAST_RESULT
    rgb = np.asarray(inputs["rgb_images_batch"], np.float32)
    assert rgb.shape == (N_CORES * B_PER_CORE, 3, 512, 512)
    B = N_CORES * B_PER_CORE
    xs = rgb.reshape(B, 3, NH, 32, 512).transpose(0, 2, 1, 3, 4)
    # centering makes the YCbCr affine offset vanish (row sums of the cb/cr
    # mix are 0 and the y row sums to 2 -> offset 2*0.5-1=0 per channel)
    xs = (np.ascontiguousarray(xs).reshape(B, NH, 96, 512)
          - np.float32(0.5)).astype(np.float16)
    # group 4 block-rows with the partition dim outermost: [B, g, 96, 4r, 512]
    xs = np.ascontiguousarray(xs.reshape(B, NH // 4, 4, 96, 512).transpose(0, 1, 3, 2, 4))
    xs = xs.reshape(B, NH // 4, 96, 4, 512)
    nc = _get_program()
    w1, w2 = _STATE["consts"]
    sort_idx = _STATE["sort_idx"]

    in_maps = [
        {
            "x": xs[c * B_PER_CORE : (c + 1) * B_PER_CORE].reshape(GROUPS, 96, 4, 512),
            "w1": w1,
            "w2": w2,
        }
        for c in range(N_CORES)
    ]
    trace = os.environ.get("KERNEL_TRACE", "0") == "1"
    res = run_bass_kernel_spmd(
        nc, in_maps, core_ids=list(range(N_CORES)), trace=trace
    )
    LAST_RESULT = res

    outs = []
    for c in range(N_CORES):
        dev = res.results[c]["out"].astype(np.float32)  # [8, 128, 3072]
        dev = dev.reshape(GROUPS // 2, 128, 8, 384).transpose(0, 2, 1, 3)
        dev = dev.reshape(ITERS, 128, 384)
        # [it=(b,br), p=(gxl,u), col=(kk, c, v)]
        a = dev.reshape(B_PER_CORE, NH, 4, 32, 4, 3, 32)  # b,br,gxl,u,kk,c,v
        a = a.transpose(0, 5, 6, 3, 1, 4, 2)  # b,c,v,u,br,kk,gxl
        a = np.ascontiguousarray(a).reshape(B_PER_CORE, 3, 1024, NH, NH)
        a = a[:, :, sort_idx, :, :]
        outs.append(a.reshape(B_PER_CORE, 3 * 1024, NH, NH))
    return np.concatenate(outs, axis=0)
